# revision 2
# baseline (speedup 1.0000x reference)
"""Winograd F(4,4) causal Conv1d (K=4) + bias + silu for TRN2, 8 NeuronCores.

Reference op: x (B=4, C_IN=2048, S=4096) fp32, weight (C_OUT=2048, C_IN, 4),
bias (C_OUT,);  out = silu(causal_conv1d(x, weight) + bias).

Sharding: data-parallel over sequence. Core c computes
out[:, :, c*512:(c+1)*512] from x[:, :, c*512-3 : c*512+512].

Algorithm: Winograd F(4,4) over the sequence dim with points
{0, +1, -1, +2, -2, +1/2, -1/2}: y = A^T [ (G w) .* (B^T d) ] per 4-output
tile -> 7 products instead of 16 per output 4-tuple, i.e. 2.29x fewer PE
cycles than direct conv (874 us -> 382 us/core roofline).

 - host: W~[t] = G w and x~[t] = B^T d, fp32 transforms cast to fp16
   (device-layout prepped); fp16 operands measure rel err ~5e-3 vs the
   2e-2 gate (fp32 PSUM accumulation, fp16 y~ evacuation).
 - device stage 1: per output-channel tile (mi in 0..15), 7 transform
   points t, each accumulating 16 k-tile matmuls [128ci x 128co] x
   [128ci, 512=(4 batch x 128 j)] into one PSUM bank (banks rotate over
   all 8; t-groups complete staggered so eviction pipelines behind PE).
 - ScalarE evacuates each bank to SBUF as fp16.
 - VectorE applies A^T via an even/odd butterfly (15 tensor_tensor +
   6 tensor_scalar ops, all fp16 2x/4x DVE modes), writing the 4 phase
   views of the y tile.
 - ScalarE bias + silu (fp32), DMA out phase-major; host re-interleaves.

Measured (device-resident reps-slope, 8 cores): 409 us/pass vs 1098 us
for the direct f32r conv baseline. TimelineSim steady state: 381.7 us
(= stage-1 PE roofline; fp16 enables fast-weight-load so LDWEIGHTS hides).
"""

import numpy as np

import concourse.bacc as bacc
import concourse.mybir as mybir
import concourse.tile as tile
from concourse.bass_utils import run_bass_kernel_spmd

P = 128

B = 4
C_IN = 2048
C_OUT = 2048
KTAPS = 4
S = 4096
N_CORES = 8
S_CHUNK = S // N_CORES          # 512
HALO = KTAPS - 1                # 3

WM = 4                          # winograd output tile size
WA = 7                          # winograd input tile / # products
NJ = S_CHUNK // WM              # 128 j-positions per core per batch
N_KI = C_IN // P                # 16
N_MI = C_OUT // P               # 16

_PTS = [0.0, 1.0, -1.0, 2.0, -2.0, 0.5, -0.5]


def _wino_mats():
    a = len(_PTS)
    V = np.array([[p ** k for k in range(a)] for p in _PTS])
    A_T = np.array([[p ** j for p in _PTS] for j in range(WM)])   # 4x7
    G = np.array([[p ** k for k in range(KTAPS)] for p in _PTS])  # 7x4
    B_T = np.linalg.inv(V).T                                      # 7x7
    return A_T, G, B_T


A_T, G_MAT, B_T = _wino_mats()


def build_wino_nc(reps=1):
    """Per-core Bass program."""
    f32 = mybir.dt.float32
    f16 = mybir.dt.float16
    silu = mybir.ActivationFunctionType.Silu
    copy_f = mybir.ActivationFunctionType.Copy

    FREE = B * NJ               # 512 matmul free dim
    YF = B * WM * NJ            # 2048 out tile free dim

    # Bacc: its compile() splits multi-wait instructions into event-semaphore
    # sequences and moves matmul waits onto ldweights (walrus rejects >1
    # sync wait per instruction otherwise).
    nc = bacc.Bacc("TRN2", target_bir_lowering=False, debug=False)

    # x~ tiles: [t*16+ki][p=ci_in, (b, j)] fp16
    xt_d = nc.dram_tensor("xt", [WA * N_KI, P, FREE], f16,
                          kind="ExternalInput").ap()
    # W~: [mi, p=ci_in, t, ki, f=co_in] fp16
    w_d = nc.dram_tensor("w", [N_MI, P, WA, N_KI, P], f16,
                         kind="ExternalInput").ap()
    bias_d = nc.dram_tensor("bias", [P, N_MI], f32, kind="ExternalInput").ap()
    # out: [mi, p=co_in, (b, r, j)] fp32, phase-major (host interleaves)
    out_d = nc.dram_tensor("out", [N_MI, P, YF], f32,
                           kind="ExternalOutput").ap()

    ps_banks = [
        nc.alloc_psum_tensor(f"psb{k}", [P, FREE], f32).ap()
        for k in range(8)
    ]

    with tile.TileContext(nc) as tc:
        with (
            tc.tile_pool(name="xtpool", bufs=1) as xtpool,
            tc.tile_pool(name="wpool", bufs=9) as wpool,
            tc.tile_pool(name="bpool", bufs=1) as bpool,
            tc.tile_pool(name="ytpool", bufs=2) as ytpool,
            tc.tile_pool(name="spool", bufs=2) as spool,
            tc.tile_pool(name="ypool", bufs=2) as ypool,
            tc.tile_pool(name="opool", bufs=2) as opool,
        ):
            bias_t = bpool.tile([P, N_MI], f32, tag="bias")
            nc.sync.dma_start(out=bias_t, in_=bias_d)

            # PE warm-up: ~4 us of matmul activity while the x~ stream is in
            # flight flips the HAM clock gate to 8/8 (2.4 GHz) before the
            # real work starts. Bank 7 is first reused by group 7 (mi=1,t=0),
            # whose start=True clears it.
            wu = bpool.tile([P, FREE], f16, tag="wu")
            nc.vector.memset(wu, 0.0)
            for _ in range(20):
                nc.tensor.matmul(ps_banks[7], wu[:, :P], wu,
                                 start=True, stop=True)

            # x~ loads ride the Activation-engine HWDGE ring so the weight
            # stream on the SP ring isn't head-of-line blocked at startup.
            xt = {}
            for t in range(WA):
                for ki in range(N_KI):
                    t_ = xtpool.tile([P, FREE], f16, tag=f"xt{t}_{ki}")
                    nc.scalar.dma_start(out=t_, in_=xt_d[t * N_KI + ki])
                    xt[t, ki] = t_

            for rep in range(reps):
                for mi in range(N_MI):
                    # stage 1: 7 transform-point groups, each 16 accumulating
                    # matmuls into one PSUM bank (rotating over all 8 banks).
                    psums = []
                    for t in range(WA):
                        w_t = wpool.tile([P, N_KI * P], f16, tag="w")
                        nc.sync.dma_start(out=w_t, in_=w_d[mi, :, t])
                        ps = ps_banks[((rep * N_MI + mi) * WA + t) % 8]
                        psums.append(ps)
                        for ki in range(N_KI):
                            nc.tensor.matmul(
                                ps, w_t[:, ki * P:(ki + 1) * P], xt[t, ki],
                                start=(ki == 0), stop=(ki == N_KI - 1),
                            )

                    # evacuate PSUM -> SBUF fp16 (ScalarE)
                    yt = ytpool.tile([P, WA, FREE], f16, tag="yt")
                    for t in range(WA):
                        nc.scalar.activation(yt[:, t], psums[t], copy_f)

                    # inverse transform A^T via even/odd butterflies (DVE).
                    # pairs: (t1,t2)=+-1, (t3,t4)=+-2, (t5,t6)=+-1/2
                    e1 = spool.tile([P, FREE], f16, tag="e1")
                    o1 = spool.tile([P, FREE], f16, tag="o1")
                    e2 = spool.tile([P, FREE], f16, tag="e2")
                    o2 = spool.tile([P, FREE], f16, tag="o2")
                    eh = spool.tile([P, FREE], f16, tag="eh")
                    oh = spool.tile([P, FREE], f16, tag="oh")
                    sc = spool.tile([P, FREE], f16, tag="sc")
                    u = spool.tile([P, FREE], f16, tag="u")
                    acc = spool.tile([P, FREE], f16, tag="acc")

                    V = nc.vector
                    V.tensor_add(e1, yt[:, 1], yt[:, 2])
                    V.tensor_sub(o1, yt[:, 1], yt[:, 2])
                    V.tensor_add(e2, yt[:, 3], yt[:, 4])
                    V.tensor_sub(o2, yt[:, 3], yt[:, 4])
                    V.tensor_add(eh, yt[:, 5], yt[:, 6])
                    V.tensor_sub(oh, yt[:, 5], yt[:, 6])

                    # y tile [p, (b, r, j)] fp16; per-r strided views
                    y_t = ypool.tile([P, B, WM, NJ], f16, tag="y")

                    # r=0: yt0 + e1 + e2 + eh
                    V.tensor_add(acc, yt[:, 0], e1)
                    V.tensor_add(u, acc, e2)
                    V.tensor_add(y_t[:, :, 0], u, eh)
                    # r=1: o1 + 2*o2 + 0.5*oh
                    V.tensor_scalar_mul(sc, o2, 2.0)
                    V.tensor_add(u, o1, sc)
                    V.tensor_scalar_mul(sc, oh, 0.5)
                    V.tensor_add(y_t[:, :, 1], u, sc)
                    # r=2: e1 + 4*e2 + 0.25*eh
                    V.tensor_scalar_mul(sc, e2, 4.0)
                    V.tensor_add(u, e1, sc)
                    V.tensor_scalar_mul(sc, eh, 0.25)
                    V.tensor_add(y_t[:, :, 2], u, sc)
                    # r=3: o1 + 8*o2 + 0.125*oh
                    V.tensor_scalar_mul(sc, o2, 8.0)
                    V.tensor_add(u, o1, sc)
                    V.tensor_scalar_mul(sc, oh, 0.125)
                    V.tensor_add(y_t[:, :, 3], u, sc)

                    # bias + silu (ScalarE), fp32 out, store
                    o_t = opool.tile([P, YF], f32, tag="o")
                    nc.scalar.activation(
                        o_t, y_t.rearrange("p b r j -> p (b r j)"), silu,
                        bias=bias_t[:, mi:mi + 1],
                    )
                    nc.sync.dma_start(out=out_d[mi], in_=o_t)
    nc.compile()
    return nc


def prep_inputs(x, weight, bias):
    """Host-side prep: pad, forward-transform, cast fp16, per-core slices."""
    xp = np.pad(np.asarray(x, np.float32), ((0, 0), (0, 0), (HALO, 0)))
    NJG = S // WM                                        # 1024 global tiles
    idx = (np.arange(NJG) * WM)[:, None] + np.arange(WA)[None, :]
    d = xp[:, :, idx]                                    # (B, C_IN, 1024, 7)
    # x~[b, ci, J, t] = sum_u B_T[t, u] d[b, ci, J, u]
    xt = (d.reshape(-1, WA) @ B_T.T.astype(np.float32)).astype(np.float16)
    xt = xt.reshape(B, C_IN, NJG, WA)

    # W~[t, co, ci] -> [mi, p=ci_in, t, ki, f=co_in]
    wt = np.einsum("tk,ock->toc", G_MAT.astype(np.float32),
                   np.asarray(weight, np.float32))
    w_dev = np.ascontiguousarray(
        wt.reshape(WA, N_MI, P, N_KI, P).transpose(1, 4, 0, 3, 2)
    ).astype(np.float16)

    bias2 = np.ascontiguousarray(
        np.asarray(bias, np.float32).reshape(N_MI, P).T)  # (P, N_MI)

    in_maps = []
    for c in range(N_CORES):
        xtc = xt[:, :, c * NJ:(c + 1) * NJ, :]           # (B, C_IN, 128, 7)
        xtc = np.ascontiguousarray(
            xtc.reshape(B, N_KI, P, NJ, WA).transpose(4, 1, 2, 0, 3)
        ).reshape(WA * N_KI, P, B * NJ)                  # [t,ki][p, (b,j)]
        in_maps.append({"xt": xtc, "w": w_dev, "bias": bias2})
    return in_maps


def assemble_out(results):
    """[mi, p, (b, r, j)] fp32 per core -> (B, C_OUT, S)."""
    outs = []
    for r in results:
        a = r["out"].reshape(N_MI, P, B, WM, NJ)
        outs.append(np.ascontiguousarray(
            a.transpose(2, 0, 1, 4, 3)).reshape(B, C_OUT, S_CHUNK))
    return np.concatenate(outs, axis=2)


def kernel(x, weight, bias):
    in_maps = prep_inputs(x, weight, bias)
    nc = build_wino_nc(reps=1)

    global LAST_RESULT
    res = run_bass_kernel_spmd(
        nc, in_maps, core_ids=list(range(N_CORES)), trace=PROFILE
    )
    LAST_RESULT = res
    return assemble_out(res.results)


PROFILE = False
LAST_RESULT = None
